# revision 23
# baseline (speedup 1.0000x reference)
"""Trainium2 Bass kernel for BondEmbedding (GNN edge embedding).

out[e, :] = concat(bond_feat[e], gaussian_smearing(|pos[i0[e]] - pos[i1[e]]|)) @ W + b

Sharding: edges split across 8 NeuronCores (embarrassingly parallel);
pos table / weights / constants replicated on every core.

v2 design (baseline was 1315 us/core, DMA-bound; v1 analysis showed the
ACT engine saturating on staging copies):
  - fp16 I/O: bond_feat shipped as fp16 (host cast) and output stored as
    fp16 (host upcast); matmuls in fp16 (1 PE pass vs 4 for f32).
  - FLIPPED matmul: out^T[o, e] = W^T @ featT with lhsT = W (static,
    SBUF-resident) and rhs = featT[f, e].  bond_feat is transposed on
    the HOST, so its [64, e] slab is used as matmul rhs directly -- no
    SBUF staging copy, no PE transpose.  Output leaves the chip
    transposed [128o, e]; the host un-transposes (free).
  - bias is folded into the gauss matmul: gauss features get a 21st
    row of ones and W_g gets a 21st row = b.
  - WINDOW0: edges are globally sorted by endpoint-0 node id on the host
    (pure permutation, un-done on the host).  Slot (p, j) of a supertile
    holds sorted edge j*128+p, so each PE chunk (fixed j) holds 128
    CONSECUTIVE sorted edges spanning < 16 node ids (avg degree 20).
    Endpoint-0 positions come from a per-chunk 16-node window: DVE
    builds one-hots, PE transposes them, and a single matmul against a
    block-diagonal [128, 32] window table yields all 8 chunks' xyz.
    This removes one of the two 256B/edge dma_gathers.
  - Endpoint 1 stays on the dma_gather path: pos table packed as
    [25000, 64] f32 (4 nodes per 256B block, idx fits int16); per edge
    gather the block, select the 16B row on DVE via a 4-wide one-hot.
  - gauss transpose: 32 PE transposes [128, 21] -> [21, 128] packed at
    psum partition bands 0 and 64 (hw allows bases {0,32,64} only),
    drained by two big [128, 1024] copies split across ACT and DVE.

dma_gather quirks handled here: indices live in partitions 0-15 wrapped
(i%16, i//16) and must be replicated to all 8 partition groups; output is
partition-fastest (gather position i -> partition i%128, slot i//128), so
the host feeds indices in transposed order; single_packet=True wedges the
SDMA (device unrecoverable) so we always pass single_packet=False.
"""

import sys

sys.path.insert(0, "/opt/trn_rl_repo")

import numpy as np

F16 = np.float16

E_TOTAL = 2_000_000
N_NODES = 100_000
IN_DIM = 64
OUT_DIM = 128
NG = 20
NG1 = NG + 1  # gauss rows + ones row (bias)
CUTOFF = 10.0
FEAT = IN_DIM + NG  # 84
N_BLOCKS = N_NODES // 4  # 25000 blocks of 4 nodes (256B each)

N_CORES = 8
SHARD = E_TOTAL // N_CORES  # 250000
K = 32                      # edges per partition per supertile
S = 128 * K                 # 4096 edges per supertile
NT = 62                     # supertiles per core
E_PC = S * NT               # 253952 edges per core (tail padded w/ repeats)
W0 = 16                     # endpoint-0 window size (nodes per 128-edge chunk)

_DELTA = CUTOFF / (NG - 1)
COEFF = -0.5 / (_DELTA * _DELTA)

_prog_cache = {}
WORK_BUFS = 3   # work-pool buffering
GATHER = True   # ablation hook: False replaces the e1 gather with memsets
PO_CHUNKS = 8   # chunks per po psum tile (8 -> 2 banks, 4 -> 1 bank)
GTPS_BUFS = 1   # gauss-transpose psum buffering


def build_program(e_pc, nt, k, repeat=1):
    """Build the per-core Bass program (identical on all cores).

    repeat>1 re-runs the whole edge sweep that many times (same inputs and
    outputs) — used only for slope-based wall-clock timing.
    """
    from concourse import bacc, mybir, tile
    from concourse.masks import make_identity

    f32 = mybir.dt.float32
    f16 = mybir.dt.float16
    i16 = mybir.dt.int16
    ALU = mybir.AluOpType
    ACT = mybir.ActivationFunctionType

    s = 128 * k
    nw = s // 16  # wrapped idx columns per supertile
    nb = k // 8   # one-hot batches (8 chunks each) per supertile

    nc = bacc.Bacc("TRN2", target_bir_lowering=False, debug=False)

    bondT = nc.dram_tensor("bondT", [nt, IN_DIM, s], f16, kind="ExternalInput")
    blk1 = nc.dram_tensor("blk1", [nt, 128, nw], i16, kind="ExternalInput")
    rem1 = nc.dram_tensor("rem1", [nt, 128, k], f32, kind="ExternalInput")
    locx = nc.dram_tensor("locx", [nt, 128, k], f32, kind="ExternalInput")
    wins = nc.dram_tensor("wins", [nt, 128, 32 * nb], f32, kind="ExternalInput")
    tab = nc.dram_tensor("tab", [N_BLOCKS, 64], f32, kind="ExternalInput")
    wbt = nc.dram_tensor("wb", [IN_DIM, OUT_DIM], f16, kind="ExternalInput")
    # wg replicated at partition bands 0 and 64 (matmul needs lhsT and rhs
    # on the same base partition; gaussT lives at bands 0 and 64)
    wgt = nc.dram_tensor("wg", [128, OUT_DIM], f16, kind="ExternalInput")
    offs = nc.dram_tensor("offs", [128, NG], f32, kind="ExternalInput")
    cand = nc.dram_tensor("cand", [128, 4], f32, kind="ExternalInput")
    cand16 = nc.dram_tensor("cand16", [128, W0], f32, kind="ExternalInput")
    out = nc.dram_tensor("out", [OUT_DIM, e_pc], f16, kind="ExternalOutput")

    with tile.TileContext(nc) as tc:
        with (
            tc.tile_pool(name="const", bufs=1) as cpool,
            tc.tile_pool(name="work", bufs=WORK_BUFS) as pool,
            tc.tile_pool(name="psum", bufs=2, space="PSUM") as ppool,
            tc.tile_pool(name="psum1", bufs=1, space="PSUM") as ppool1,
            tc.tile_pool(name="psumg", bufs=GTPS_BUFS, space="PSUM") as ppoolg,
        ):
            wb_sb = cpool.tile([IN_DIM, OUT_DIM], f16, tag="wb")
            nc.sync.dma_start(out=wb_sb[:], in_=wbt[:, :])
            wg_sb = cpool.tile([128, OUT_DIM], f16, tag="wg")
            nc.sync.dma_start(out=wg_sb[:], in_=wgt[:, :])
            offs_sb = cpool.tile([128, NG], f32, tag="offs")
            nc.sync.dma_start(out=offs_sb[:], in_=offs[:, :])
            cand_sb = cpool.tile([128, 4], f32, tag="cand")
            nc.sync.dma_start(out=cand_sb[:], in_=cand[:, :])
            cand16_sb = cpool.tile([128, W0], f32, tag="cand16")
            nc.sync.dma_start(out=cand16_sb[:], in_=cand16[:, :])
            identh = cpool.tile([128, 128], f16, tag="identh")
            make_identity(nc, identh[:])
            identf = cpool.tile([128, 128], f32, tag="identf")
            make_identity(nc, identf[:])

            for t in range(nt * repeat):
                t = t % nt
                e0 = t * s

                # --- endpoint 0: windowed one-hot gather ------------------
                lt = pool.tile([128, k], f32, tag="lt")
                nc.sync.dma_start(out=lt[:], in_=locx[t, :, :])
                ws = pool.tile([128, 32 * nb], f32, tag="ws")
                nc.sync.dma_start(out=ws[:], in_=wins[t, :, :])
                # one-hots for all chunks: oh[p, (j, v)] = (locx[p,j] == v)
                oh = pool.tile([128, W0 * k], f32, tag="oh")
                nc.vector.tensor_tensor(
                    out=oh[:].rearrange("p (j v) -> p j v", v=W0),
                    in0=lt[:].unsqueeze(2).to_broadcast([128, k, W0]),
                    in1=cand16_sb[:].unsqueeze(1).to_broadcast([128, k, W0]),
                    op=ALU.is_equal,
                )
                ohT_ps = ppool1.tile([128, 128 * nb], f32, tag="ohT_ps")
                for b in range(nb):
                    nc.tensor.transpose(
                        out=ohT_ps[:, 128 * b : 128 * (b + 1)],
                        in_=oh[:, 128 * b : 128 * (b + 1)],
                        identity=identf[:],
                    )
                ohT = pool.tile([128, 128 * nb], f32, tag="ohT")
                nc.scalar.activation(out=ohT[:], in_=ohT_ps[:], func=ACT.Copy)
                # ws[:, 32b:32b+32] is block-diagonal: row (g,v) has window
                # g's xyz at cols 4g..4g+3, so one full-K matmul per batch
                # separates the 8 chunks.
                pos_ps = ppool1.tile([128, 32 * nb], f32, tag="pos_ps")
                for b in range(nb):
                    nc.tensor.matmul(
                        out=pos_ps[:, 32 * b : 32 * (b + 1)],
                        lhsT=ohT[:, 128 * b : 128 * (b + 1)],
                        rhs=ws[:, 32 * b : 32 * (b + 1)],
                        start=True,
                        stop=True,
                    )
                rr0 = pool.tile([128, 4 * k], f32, tag="r0")
                nc.scalar.activation(out=rr0[:], in_=pos_ps[:], func=ACT.Copy)

                # --- endpoint 1: gather pos block, select row -------------
                if GATHER:
                    bt = pool.tile([128, nw], i16, tag="blk1")
                    nc.sync.dma_start(out=bt[:], in_=blk1[t, :, :])
                    gth = pool.tile([128, (s // 128) * 64], f32, tag="gth1")
                    nc.gpsimd.dma_gather(
                        out_ap=gth[:].rearrange("p (k c) -> p k c", c=64),
                        in_ap=tab[:, :],
                        idxs_ap=bt[:],
                        num_idxs=s,
                        num_idxs_reg=s,
                        elem_size=64,
                        single_packet=False,
                        queue_num=0,
                    )
                    rt = pool.tile([128, k], f32, tag="rem1")
                    nc.sync.dma_start(out=rt[:], in_=rem1[t, :, :])
                    oh1 = pool.tile([128, 4 * k], f32, tag="oh1")
                    nc.vector.tensor_tensor(
                        out=oh1[:].rearrange("p (k m) -> p k m", m=4),
                        in0=rt[:].unsqueeze(2).to_broadcast([128, k, 4]),
                        in1=cand_sb[:].unsqueeze(1).to_broadcast([128, k, 4]),
                        op=ALU.is_equal,
                    )
                    tmp = pool.tile([128, 16 * k], f32, tag="tmp1")
                    # gth element (k, m, v): edge chunk k, node-slot m, 16-f32
                    # row v; want [p][k][c=v<4][m] ordering with m innermost
                    gv = gth[:].rearrange("p (k m v) -> p k v m", m=4, v=16)
                    nc.vector.tensor_tensor(
                        out=tmp[:].rearrange("p (k c m) -> p k c m", c=4, m=4),
                        in0=gv[:, :, 0:4, :],
                        in1=oh1[:]
                        .rearrange("p (k m) -> p k m", m=4)
                        .unsqueeze(2)
                        .to_broadcast([128, k, 4, 4]),
                        op=ALU.mult,
                    )
                    rr1 = pool.tile([128, 4 * k], f32, tag="r1")
                    nc.vector.tensor_reduce(
                        out=rr1[:].rearrange("p (k c) -> p k c", c=4),
                        in_=tmp[:].rearrange("p (k c m) -> p k c m", c=4, m=4),
                        axis=mybir.AxisListType.X,
                        op=ALU.add,
                    )
                else:
                    rr1 = pool.tile([128, 4 * k], f32, tag="r1")
                    nc.vector.memset(rr1[:], 2.0)

                # --- distance -> d = sqrt(dist2) via exp(0.5*ln(.)) -------
                diff = pool.tile([128, 4 * k], f32, tag="diff")
                nc.vector.tensor_tensor(
                    out=diff[:], in0=rr0[:], in1=rr1[:], op=ALU.subtract
                )
                sq = pool.tile([128, 4 * k], f32, tag="sq")
                nc.vector.tensor_tensor(out=sq[:], in0=diff[:], in1=diff[:], op=ALU.mult)
                dist2 = pool.tile([128, k], f32, tag="dist2")
                nc.vector.tensor_reduce(
                    out=dist2[:],
                    in_=sq[:].rearrange("p (k c) -> p k c", c=4),
                    axis=mybir.AxisListType.X,
                    op=ALU.add,
                )
                # clamp so ln() stays finite; exp(0.5*ln(1e-35)) ~ 3e-18 ~ 0
                nc.vector.tensor_scalar_max(out=dist2[:], in0=dist2[:], scalar1=1e-35)
                d = pool.tile([128, k], f32, tag="d")
                nc.scalar.activation(out=d[:], in_=dist2[:], func=ACT.Ln)
                nc.scalar.activation(out=d[:], in_=d[:], func=ACT.Exp, scale=0.5)

                # --- gauss tile [128, k*21] fp16 (21st col = ones) --------
                gauss = pool.tile([128, NG1 * k], f16, tag="gauss")
                gaussv = gauss[:].rearrange("p (k f) -> p k f", f=NG1)
                nc.vector.memset(gaussv[:, :, NG:NG1], 1.0)
                u = pool.tile([128, NG * k], f32, tag="u")
                uv = u[:].rearrange("p (k g) -> p k g", g=NG)
                nc.vector.tensor_tensor(
                    out=uv,
                    in0=d[:].unsqueeze(2).to_broadcast([128, k, NG]),
                    in1=offs_sb[:].unsqueeze(1).to_broadcast([128, k, NG]),
                    op=ALU.subtract,
                )
                usq = pool.tile([128, NG * k], f32, tag="usq")
                nc.vector.tensor_tensor(out=usq[:], in0=u[:], in1=u[:], op=ALU.mult)
                nc.scalar.activation(
                    out=gaussv[:, :, 0:NG],
                    in_=usq[:].rearrange("p (k g) -> p k g", g=NG),
                    func=ACT.Exp,
                    scale=COEFF,
                )

                # --- gaussT: 32 transposes into 2 psum bands, 2 big drains
                gtps = ppoolg.tile([128, 128 * (k // 2)], f16, tag="gtps")
                for kk in range(k):
                    band, col = (0, kk) if kk < k // 2 else (64, kk - k // 2)
                    nc.tensor.transpose(
                        out=gtps[band : band + NG1, 128 * col : 128 * (col + 1)],
                        in_=gauss[:, NG1 * kk : NG1 * (kk + 1)],
                        identity=identh[:],
                    )
                gaussT = pool.tile([128, 128 * (k // 2)], f16, tag="gaussT")
                half = 128 * (k // 4)
                nc.scalar.activation(
                    out=gaussT[:, 0:half], in_=gtps[:, 0:half], func=ACT.Copy
                )
                nc.vector.tensor_copy(gaussT[:, half:], gtps[:, half:])

                # --- flipped matmuls: po[o, e] = Wb^T bondT + Wg^T gaussT
                bT = pool.tile([IN_DIM, s], f16, tag="bT")
                nc.sync.dma_start(out=bT[:], in_=bondT[t, :, :])
                outsb = pool.tile([128, s], f16, tag="outsb")
                po = None
                for kk in range(k):
                    m = kk % PO_CHUNKS
                    if m == 0:
                        po = ppool.tile([128, 128 * PO_CHUNKS], f32, tag="po")
                    nc.tensor.matmul(
                        out=po[:, 128 * m : 128 * (m + 1)],
                        lhsT=wb_sb[:],
                        rhs=bT[:, 128 * kk : 128 * (kk + 1)],
                        start=True,
                        stop=False,
                    )
                    band, col = (0, kk) if kk < k // 2 else (64, kk - k // 2)
                    nc.tensor.matmul(
                        out=po[:, 128 * m : 128 * (m + 1)],
                        lhsT=wg_sb[band : band + NG1, :],
                        rhs=gtps_view(gaussT, band, col, NG1),
                        start=False,
                        stop=True,
                    )
                    if m == PO_CHUNKS - 1:
                        q = kk // PO_CHUNKS
                        w = 128 * PO_CHUNKS
                        dst = outsb[:, w * q : w * (q + 1)]
                        if q % 2 == 0:
                            nc.scalar.activation(out=dst, in_=po[:], func=ACT.Copy)
                        else:
                            nc.vector.tensor_copy(dst, po[:])

                # store issued from the ACT queue: input loads (SP queue)
                # must not stall behind the store's sem wait at queue head
                nc.scalar.dma_start(out=out[:, e0 : e0 + s], in_=outsb[:])

    nc.compile()
    return nc


def gtps_view(gaussT, band, col, ng1):
    return gaussT[band : band + ng1, 128 * col : 128 * (col + 1)]


def get_program(e_pc=E_PC, nt=NT, k=K):
    key = (e_pc, nt, k)
    if key not in _prog_cache:
        _prog_cache[key] = build_program(e_pc, nt, k)
    return _prog_cache[key]


def _gather_inputs(idx, nt, k):
    """blk (wrapped+replicated int16 block idx) and rem (f32 idx%4) slabs.

    idx is in SLAB ROW ORDER: flat position t*s + p*k + j = slot (t, p, j).
    """
    s = 128 * k
    nw = s // 16
    # gather-position i covers local edge slot (i%128)*k + i//128
    ii = np.arange(s)
    perm = (ii % 128) * k + (ii // 128)
    blk = (idx >> 2).astype(np.int16).reshape(nt, s)[:, perm]  # [nt, s]
    wrapped = blk.reshape(nt, nw, 16).transpose(0, 2, 1)  # [nt, 16, nw]
    blk_t = np.broadcast_to(wrapped[:, None, :, :], (nt, 8, 16, nw)).reshape(
        nt, 128, nw
    )
    rem = (idx & 3).astype(np.float32).reshape(nt, 128, k)
    return np.ascontiguousarray(blk_t), np.ascontiguousarray(rem)


def make_in_maps(bond_feat, bond_index, pos_nodes, W, b, e_pc=E_PC, nt=NT, k=K):
    """Shard + sort the full problem into per-core input maps.

    Edges are globally sorted by endpoint-0 node id; core c takes sorted
    positions [c*SHARD, (c+1)*SHARD), padded to e_pc by repeating the last
    edge (padded rows are computed redundantly and discarded).

    Two edge orders coexist per core:
      - bondT / out use plain core-sorted order (supertile t, flat i):
        edge t*s + i, i.e. chunk kk = edges [t*s + 128*kk, +128).
      - the [128, k]-shaped tiles (locx, blk1, rem1, d, ...) use slot
        (t, p, j) = sorted edge t*s + j*128 + p (chunk j on free dim).

    Returns (in_maps, sort_perm); outputs un-permute via sort_perm only
    (the out slab is already in core-sorted order).
    """
    s = 128 * k
    bond_feat = np.asarray(bond_feat)
    idx0_all = np.ascontiguousarray(np.asarray(bond_index[0]).astype(np.int64))
    idx1_all = np.ascontiguousarray(np.asarray(bond_index[1]).astype(np.int32))
    pos_nodes = np.asarray(pos_nodes, dtype=np.float32)

    sort_perm = np.argsort(idx0_all, kind="stable")

    # slab-row r (slot order) <-> core sorted-edge order[r]
    r = np.arange(e_pc)
    within = r % s
    order = (r // s) * s + (within % k) * 128 + within // k

    tab = np.zeros((N_NODES, 16), dtype=np.float32)
    tab[:, :3] = pos_nodes
    tab = tab.reshape(N_BLOCKS, 64)
    pos_pad = np.concatenate(
        [pos_nodes, np.zeros((W0, 3), np.float32)], axis=0
    )  # windows may read past the last node id

    offs_row = np.linspace(0.0, CUTOFF, NG, dtype=np.float32)
    offs_bcast = np.ascontiguousarray(np.broadcast_to(offs_row, (128, NG)))
    cand = np.ascontiguousarray(
        np.broadcast_to(np.arange(4, dtype=np.float32), (128, 4))
    )
    cand16 = np.ascontiguousarray(
        np.broadcast_to(np.arange(W0, dtype=np.float32), (128, W0))
    )
    W = np.asarray(W, dtype=np.float32)
    wb_np = np.ascontiguousarray(W[:IN_DIM]).astype(F16)
    wg21 = np.concatenate(
        [W[IN_DIM:FEAT], np.asarray(b, np.float32)[None, :]], axis=0
    ).astype(F16)
    wg_np = np.zeros((128, OUT_DIM), dtype=F16)
    wg_np[0:NG1] = wg21
    wg_np[64 : 64 + NG1] = wg21

    in_maps = []
    for c in range(N_CORES):
        start = c * SHARD
        sel = sort_perm[start : start + SHARD]
        sel = np.concatenate([sel, np.repeat(sel[-1:], e_pc - SHARD)])
        se = sel[order]  # original edge id for each slab row (slot order)

        i0 = idx0_all[se]
        i1 = idx1_all[se]

        # per-chunk windows over i0; chunk (t, j) = slab rows with that (t, j)
        i0_c = i0.reshape(nt, 128, k)  # [t, p, j]
        base = i0_c.min(axis=1)  # [t, j]
        span = i0_c.max(axis=1) - base
        assert span.max() < W0, f"window overflow: span {span.max()} >= {W0}"
        lx = (i0_c - base[:, None, :]).astype(np.float32)  # [t, 128, j]

        # wins[t, 16g+v, 32b + 4g+c] = pos_pad[base[t, 8b+g] + v][c]
        nb = k // 8
        g_idx = np.arange(128) // W0  # g within batch
        v_idx = np.arange(128) % W0
        wins = np.zeros((nt, 128, 32 * nb), dtype=np.float32)
        for bb in range(nb):
            node = base[:, 8 * bb + g_idx] + v_idx  # [t, 128]
            xyz = pos_pad[node]  # [t, 128, 3]
            for cc in range(3):
                wins[:, np.arange(128), 32 * bb + 4 * g_idx + cc] = xyz[:, :, cc]

        b1, r1 = _gather_inputs(i1, nt, k)
        # bondT: [nt, 64, s] = per-supertile transpose of core-sorted rows
        bslab = bond_feat[sel].astype(F16).reshape(nt, s, IN_DIM)
        bondT = np.ascontiguousarray(bslab.transpose(0, 2, 1))
        in_maps.append(
            {
                "bondT": bondT,
                "blk1": b1,
                "rem1": r1,
                "locx": np.ascontiguousarray(lx),
                "wins": wins,
                "tab": tab,
                "wb": wb_np,
                "wg": wg_np,
                "offs": offs_bcast,
                "cand": cand,
                "cand16": cand16,
            }
        )
    return in_maps, sort_perm


def kernel(bond_feat, bond_index, pos_nodes, W, b):
    from concourse.bass_utils import run_bass_kernel_spmd

    nc = get_program()
    in_maps, sort_perm = make_in_maps(bond_feat, bond_index, pos_nodes, W, b)
    res = run_bass_kernel_spmd(nc, in_maps, core_ids=list(range(N_CORES)))

    full = np.empty((E_TOTAL, OUT_DIM), dtype=np.float32)
    for c in range(N_CORES):
        slab = np.asarray(res.results[c]["out"])  # [128o, e_pc], fp16
        full[sort_perm[c * SHARD : (c + 1) * SHARD]] = (
            slab[:, :SHARD].T.astype(np.float32)
        )
    return full


def reference_numpy(bond_feat, bond_index, pos_nodes, W, b):
    """Pure-numpy oracle for local testing."""
    diff = pos_nodes[bond_index[0]] - pos_nodes[bond_index[1]]
    dist = np.sqrt(np.sum(diff * diff, axis=-1))
    offs_row = np.linspace(0.0, CUTOFF, NG, dtype=np.float32)
    dd = dist[:, None] - offs_row[None, :]
    gauss = np.exp(COEFF * dd * dd)
    feat = np.concatenate([bond_feat, gauss.astype(np.float32)], axis=-1)
    return feat @ W + b
